# revision 1
# baseline (speedup 1.0000x reference)
"""Trainium2 Bass kernel for nn_DAMSoftmax (sub-center ArcFace loss, model-parallel softmax CE).

Contract: kernel(**inputs) takes FULL inputs {input:(1024,128) f32, factor:(1024,1) f32,
label:(1024,) int32, weight:(16,128,10000) f32} and returns (cls_loss, prec1) scalars,
matching the reference.

Strategy:
  - Shard OUT=10000 classes across 8 cores (1250 each).
  - Host: L2-normalize input rows and weight columns (cheap vs the 82MB matmul),
    upload fp16 xnT (128,1024) + per-core fp16 weight shard (128, 16*1250).
  - Device (per core): for each k-plane, matmul xnT_bt^T @ w_k -> PSUM (fp32),
    running elementwise max over the 16 sub-center planes into an fp16 accumulator
    (eviction split between ScalarE copies + VectorE fp16 2x merges), then per-row
    max (VectorE reduce) and sum(exp(S*cos - S*rowmax)) via ScalarE Exp with
    accum_out. Device outputs per core: (128,16) = [rowmax x8 btiles, sumexp x8].
  - Host: exact cross-core log-sum-exp, label-column margin replacement (label
    cosines recomputed on host in fp32/fp16 to match device rounding), top-1
    accuracy with exact fallback for ambiguous rows.
"""

import math
import numpy as np

S = 64.0
MARGIN = 0.5
C = 1.5
K = 16
EPS = 1e-6
IN = 128
OUT = 10000
B = 1024
NCORES = 8
OSH = OUT // NCORES  # 1250 classes per core
NBT = B // 128       # 8 batch tiles

# Eviction split: planes evicted by ScalarE (copy->fp16, then DVE 2x merge)
# vs planes merged by VectorE directly from PSUM (1x). k=0 initializes acc via ACT copy.
DVE_DIRECT_KS = ()
GPS_MERGE_KS = ()  # ACT-evicted planes whose fp16 merge runs on GpSimd instead of DVE
FUSE_ROWMAX = False  # InstTensorTensorReduce crashes at runtime on this terminal


def _build_nc_wide(repeat=1, n_act=12, tmpw_bufs=3, psum_bufs=2):
    """Wide-merge structure: ACT evicts each k-plane (per bt) into a wide
    (128, NBT*OSH) fp16 tile; DVE merges whole wide tiles (one op per k).
    DVE-direct planes chain into a second wide accumulator via per-(k,bt)
    PSUM reads, filling DVE slack during ACT rounds."""
    import concourse.bacc as bacc
    import concourse.tile as tile
    from concourse import mybir

    f32 = mybir.dt.float32
    f16 = mybir.dt.float16
    W = NBT * OSH

    act_ks = tuple(range(n_act))          # evicted by ACT (k=0 writes accw directly)
    dve_ks = tuple(range(n_act, K))       # DVE-direct from PSUM into accd

    nc = bacc.Bacc(
        "TRN2", target_bir_lowering=False, debug=False, num_devices=NCORES
    )
    xnT_d = nc.declare_dram_parameter("xnT", (IN, B), f16, isOutput=False)
    w_d = nc.declare_dram_parameter("w", (IN, K * OSH), f16, isOutput=False)
    out_d = nc.declare_dram_parameter("out", (128, 16), f32, isOutput=True)

    with tile.TileContext(nc) as tc:
        with (
            tc.tile_pool(name="consts", bufs=1) as cpool,
            tc.tile_pool(name="wpool", bufs=1) as wpool,
            tc.tile_pool(name="psum", bufs=psum_bufs, space="PSUM") as ppool,
            tc.tile_pool(name="accp", bufs=1) as accpool,
            tc.tile_pool(name="tmpp", bufs=tmpw_bufs) as tmppool,
            tc.tile_pool(name="stats", bufs=1) as statpool,
        ):
            xnT_sb = cpool.tile([IN, B], f16)
            nc.sync.dma_start(xnT_sb[:, :], xnT_d[:, :])

            w_sb = [wpool.tile([IN, OSH], f16, tag=f"w{k}", name=f"w{k}") for k in range(K)]
            for k in range(K):
                nc.sync.dma_start(w_sb[k][:, :], w_d[:, k * OSH:(k + 1) * OSH])

            accw = accpool.tile([128, W], f16, tag="accw")
            accd = accpool.tile([128, W], f16, tag="accd") if dve_ks else None
            out_sb = statpool.tile([128, 16], f32)
            bias_row = statpool.tile([128, NBT], f32, tag="bias")

            mm_chunks = [(0, 512), (512, 512), (1024, OSH - 1024)]

            for _rep in range(repeat):
                # interleave: ACT plane, then (if any left) a DVE plane, so both
                # engines have work each round; Tile reorders within deps anyway.
                order = []
                ai, di = list(act_ks), list(dve_ks)
                while ai or di:
                    if ai:
                        order.append(ai.pop(0))
                    if di:
                        order.append(di.pop(0))
                for k in order:
                    for bt in range(NBT):
                        ps = ppool.tile([128, OSH], f32, tag="ps", name=f"ps_{_rep}_{k}_{bt}")
                        for (c0, cn) in mm_chunks:
                            nc.tensor.matmul(
                                ps[:, c0:c0 + cn],
                                xnT_sb[:, bt * 128:(bt + 1) * 128],
                                w_sb[k][:, c0:c0 + cn],
                                start=True,
                                stop=True,
                            )
                        sl = slice(bt * OSH, (bt + 1) * OSH)
                        if k in act_ks:
                            if k == 0:
                                nc.scalar.copy(accw[:, sl], ps[:, :])
                            else:
                                tmpw = tmppool.tile([128, W], f16, tag="tmpw", name=f"tmpw_{_rep}_{k}")                                 if bt == 0 else tmpw
                                nc.scalar.copy(tmpw[:, sl], ps[:, :])
                        else:
                            if k == min(dve_ks):
                                nc.vector.tensor_copy(accd[:, sl], ps[:, :])
                            else:
                                nc.vector.tensor_max(accd[:, sl], accd[:, sl], ps[:, :])
                    if k in act_ks and k != 0:
                        nc.vector.tensor_max(accw[:, :], accw[:, :], tmpw[:, :])

                if accd is not None:
                    nc.vector.tensor_max(accw[:, :], accw[:, :], accd[:, :])
                # wide rowmax: (128, NBT, OSH) -> (128, NBT)
                nc.vector.reduce_max(
                    out_sb[:, 0:NBT], accw.rearrange("p (n o) -> p n o", n=NBT),
                    axis=mybir.AxisListType.X,
                )
                nc.vector.tensor_scalar_mul(bias_row[:, :], out_sb[:, 0:NBT], -S)
                for bt in range(NBT):
                    sl = slice(bt * OSH, (bt + 1) * OSH)
                    nc.scalar.activation(
                        accw[:, sl],
                        accw[:, sl],
                        mybir.ActivationFunctionType.Exp,
                        bias=bias_row[:, bt:bt + 1],
                        scale=S,
                        accum_out=out_sb[:, 8 + bt:9 + bt],
                    )

            nc.sync.dma_start(out_d[:, :], out_sb[:, :])
    nc.compile()
    return nc


def _build_nc(repeat=1, dve_ks=None, gps_ks=None, fuse_rowmax=None, pe_only=False, skip_tail=False, tmp_bufs=4, psum_bufs=2, bt_outer=False, two_acc=False):
    import concourse.bacc as bacc
    import concourse.tile as tile
    from concourse import mybir

    f32 = mybir.dt.float32
    f16 = mybir.dt.float16
    if dve_ks is None:
        dve_ks = DVE_DIRECT_KS
    if gps_ks is None:
        gps_ks = GPS_MERGE_KS
    if fuse_rowmax is None:
        fuse_rowmax = FUSE_ROWMAX

    nc = bacc.Bacc(
        "TRN2", target_bir_lowering=False, debug=False, num_devices=NCORES
    )
    xnT_d = nc.declare_dram_parameter("xnT", (IN, B), f16, isOutput=False)
    w_d = nc.declare_dram_parameter("w", (IN, K * OSH), f16, isOutput=False)
    out_d = nc.declare_dram_parameter("out", (128, 16), f32, isOutput=True)

    with tile.TileContext(nc) as tc:
        with (
            tc.tile_pool(name="consts", bufs=1) as cpool,
            tc.tile_pool(name="wpool", bufs=1) as wpool,
            tc.tile_pool(name="psum", bufs=psum_bufs, space="PSUM") as ppool,
            tc.tile_pool(name="accp", bufs=1) as accpool,
            tc.tile_pool(name="tmpp", bufs=tmp_bufs) as tmppool,
            tc.tile_pool(name="stats", bufs=1) as statpool,
        ):
            xnT_sb = cpool.tile([IN, B], f16)
            nc.sync.dma_start(xnT_sb[:, :], xnT_d[:, :])

            w_sb = [wpool.tile([IN, OSH], f16, tag=f"w{k}", name=f"w{k}") for k in range(K)]
            for k in range(K):
                nc.sync.dma_start(w_sb[k][:, :], w_d[:, k * OSH:(k + 1) * OSH])

            acc = None if pe_only else [accpool.tile([128, OSH], f16, tag=f"acc{bt}", name=f"acc{bt}") for bt in range(NBT)]
            accd = None
            if two_acc and not pe_only:
                accd = [accpool.tile([128, OSH], f16, tag=f"accd{bt}", name=f"accd{bt}") for bt in range(NBT)]
            out_sb = statpool.tile([128, 16], f32)
            bias_col = statpool.tile([128, NBT], f32, tag="bias")

            mm_chunks = [(0, 512), (512, 512), (1024, OSH - 1024)]

            for _rep in range(repeat):
                loop_iter = (
                    [(k, bt) for bt in range(NBT) for k in range(K)]
                    if bt_outer else
                    [(k, bt) for k in range(K) for bt in range(NBT)]
                )
                tail_done = set()
                def emit_tail(bt):
                    if not fuse_rowmax:
                        nc.vector.reduce_max(
                            out_sb[:, bt:bt + 1], acc[bt][:, :], axis=mybir.AxisListType.X
                        )
                    nc.vector.tensor_scalar_mul(
                        bias_col[:, bt:bt + 1], out_sb[:, bt:bt + 1], -S
                    )
                    nc.scalar.activation(
                        acc[bt][:, :],
                        acc[bt][:, :],
                        mybir.ActivationFunctionType.Exp,
                        bias=bias_col[:, bt:bt + 1],
                        scale=S,
                        accum_out=out_sb[:, 8 + bt:9 + bt],
                    )
                for (k, bt) in loop_iter:
                    if True:
                        ps = ppool.tile([128, OSH], f32, tag="ps", name=f"ps_{_rep}_{k}_{bt}")
                        lhsT = xnT_sb[:, bt * 128:(bt + 1) * 128]
                        for (c0, cn) in mm_chunks:
                            nc.tensor.matmul(
                                ps[:, c0:c0 + cn],
                                lhsT,
                                w_sb[k][:, c0:c0 + cn],
                                start=True,
                                stop=True,
                            )
                        if pe_only:
                            continue
                        if k == 0:
                            nc.scalar.copy(acc[bt][:, :], ps[:, :])
                        elif k in dve_ks:
                            if two_acc:
                                tgt = accd[bt]
                                if k == min(dve_ks):
                                    nc.vector.tensor_copy(tgt[:, :], ps[:, :])
                                else:
                                    nc.vector.tensor_max(tgt[:, :], tgt[:, :], ps[:, :])
                            elif fuse_rowmax and k == K - 1:
                                nc.vector.tensor_tensor_reduce(
                                    acc[bt][:, :], ps[:, :], acc[bt][:, :],
                                    1.0, -2.0,
                                    mybir.AluOpType.max, mybir.AluOpType.max,
                                    accum_out=out_sb[:, bt:bt + 1],
                                )
                            else:
                                nc.vector.tensor_max(acc[bt][:, :], acc[bt][:, :], ps[:, :])
                        else:
                            tmp = tmppool.tile([128, OSH], f16, tag="tmp", name=f"tmp_{_rep}_{k}_{bt}")
                            nc.scalar.copy(tmp[:, :], ps[:, :])
                            eng = nc.gpsimd if k in gps_ks else nc.vector
                            if fuse_rowmax and k == K - 1:
                                nc.vector.tensor_tensor_reduce(
                                    acc[bt][:, :], tmp[:, :], acc[bt][:, :],
                                    1.0, -2.0,
                                    mybir.AluOpType.max, mybir.AluOpType.max,
                                    accum_out=out_sb[:, bt:bt + 1],
                                )
                            else:
                                eng.tensor_max(acc[bt][:, :], acc[bt][:, :], tmp[:, :])

                        if k == K - 1 and not (pe_only or skip_tail):
                            if two_acc and dve_ks:
                                nc.vector.tensor_max(acc[bt][:, :], acc[bt][:, :], accd[bt][:, :])
                            emit_tail(bt)

            nc.sync.dma_start(out_d[:, :], out_sb[:, :])
    nc.compile()
    return nc


_NC_CACHE = {}


def _get_nc(repeat=1):
    key = f"nc{repeat}"
    if key not in _NC_CACHE:
        _NC_CACHE[key] = _build_nc(repeat)
    return _NC_CACHE[key]


def _l2norm_np(x, axis):
    n = np.linalg.norm(x, axis=axis, keepdims=True)
    return x / np.maximum(n, 1e-12)


def kernel(input, factor, label, weight):
    from concourse.bass_utils import run_bass_kernel_spmd

    input = np.asarray(input, dtype=np.float32)
    factor = np.asarray(factor, dtype=np.float32)
    label = np.asarray(label)
    weight = np.asarray(weight, dtype=np.float32)

    # ---- host preprocessing ----
    xn = _l2norm_np(input, axis=1)                       # (B, IN) fp32
    wn = _l2norm_np(weight, axis=1)                      # (K, IN, OUT) fp32
    xnT16 = np.ascontiguousarray(xn.T).astype(np.float16)  # (IN, B)

    in_maps = []
    for c in range(NCORES):
        sh = wn[:, :, c * OSH:(c + 1) * OSH]             # (K, IN, OSH)
        w_dev = np.ascontiguousarray(
            sh.transpose(1, 0, 2).reshape(IN, K * OSH)
        ).astype(np.float16)                             # (IN, K*OSH), k-major planes
        in_maps.append({"xnT": xnT16, "w": w_dev})

    nc = _get_nc()
    res = run_bass_kernel_spmd(nc, in_maps, list(range(NCORES)))
    outs = [np.asarray(res.results[c]["out"]) for c in range(NCORES)]  # (128,16) each

    # lmax/lsum per core, reassembled to (NCORES, B)
    lmax = np.stack([o[:, 0:8].T.reshape(B) for o in outs])   # cos units
    lsum = np.stack([o[:, 8:16].T.reshape(B) for o in outs])

    # ---- host: exact label-column logits ----
    xn16 = xnT16.T.astype(np.float32)                   # device-rounded xn (B, IN)
    wn16 = wn.astype(np.float16).astype(np.float32)     # device-rounded weights
    # label-column cosines as the device computed them (fp16 inputs, fp32 accum)
    wl16 = wn16[:, :, label]                            # (K, IN, B)
    v_dev = np.einsum("bf,kfb->kb", xn16, wl16, optimize=True).max(axis=0)  # (B,)
    v16 = v_dev.astype(np.float16).astype(np.float64)   # matches fp16 acc rounding
    # true fp32 label cosines (for the reference-accurate margined logit)
    wl = wn[:, :, label]                                # (K, IN, B)
    v_true = np.einsum("bf,kfb->kb", xn.astype(np.float32), wl, optimize=True).max(axis=0)

    # margined label logit, replicating the reference formula exactly
    func_a = (np.power(C, factor[:, 0] / 12.0) * MARGIN).astype(np.float32)  # (B,)
    threshold = (math.pi - func_a).astype(np.float32)
    theta = np.arccos(np.clip(v_true, -1.0 + EPS, 1.0 - EPS).astype(np.float32))
    sel = ~(theta > threshold)  # margin applied iff theta <= threshold
    theta_adj = np.where(sel, theta + func_a, theta)
    l_true = (np.cos(theta_adj) * S).astype(np.float64)  # final label logit (B,)

    # ---- host: cross-core LSE with label-column replacement (fp64) ----
    lmax64 = lmax.astype(np.float64) * S                 # logits units (NCORES, B)
    lsum64 = lsum.astype(np.float64)
    R = lmax64.max(axis=0)                               # (B,) global rowmax (unmargined)
    Z = (np.exp(lmax64 - R[None, :]) * lsum64).sum(axis=0)
    Zp = Z - np.exp(S * v16 - R) + np.exp(l_true - R)
    lse = R + np.log(Zp)
    loss = np.mean(lse - l_true)

    # ---- host: top-1 accuracy ----
    # pred == label iff the (margined) label logit beats every other column.
    # Device rowmax R/S (cos units) includes the unmargined label col; the margin
    # only lowers the label logit. Guard band covers fp16 rounding (~6e-4 cos).
    Rc = R / S                                           # global rowmax, cos units
    guard = 2e-3
    safe_not_label = (v16 < Rc - guard) & (l_true / S < Rc - guard)
    n_correct = 0
    ambiguous = np.nonzero(~safe_not_label)[0]
    if len(ambiguous) > 0:
        # exact fallback: full-row recompute in fp32 (reference-exact math)
        for b in ambiguous:
            cos_b = np.einsum("f,kfo->ko", xn[b].astype(np.float32),
                              wn.astype(np.float32), optimize=True).max(axis=0)
            th = np.arccos(np.clip(cos_b, -1.0 + EPS, 1.0 - EPS))
            fa = func_a[b]
            one = np.zeros(OUT, dtype=bool)
            one[label[b]] = True
            sel_b = one & ~(th > (math.pi - fa))
            logits_b = np.cos(np.where(sel_b, th + fa, th)) * S
            if logits_b.argmax() == label[b]:
                n_correct += 1
    prec1 = n_correct / B * 100.0

    return np.float32(loss), np.float32(prec1)

